# revision 6
# baseline (speedup 1.0000x reference)
"""Trainium2 Bass kernel for the NeRF MoE-routing module.

Strategy
--------
Data-parallel over points N=131072 across 8 NeuronCores (16384 points/core).
On-chip layout is transposed: activations live as [features, points] with
features on SBUF partitions, so every linear layer is one PE matmul
(out = lhsT.T @ rhs) and layers chain without transposes.

Host-side folding (all free — done once in numpy):
  * part features pf, their attention projections k/v, and the per-head
    block-diagonal attention matrices (Kblk/Vblk) are precomputed.
  * the q-projection is folded through Win into a single [din,32] matrix.
  * LayerNorm mean subtraction is folded into centered weights (P = I-11^T/128)
    so the kernel only computes the rstd scaling.
  * the view embedding (sin/cos) is precomputed into a [39, N] feature matrix.
Matmuls run in float32r (full-speed fp32-storage mode, ~1e-4 relative error).
"""
import os
import sys

sys.path.insert(0, "/opt/trn_rl_repo")

import numpy as np

N_TOTAL = 131072
NCORES = 8
NP_CORE = N_TOTAL // NCORES          # 16384 points per core
TILE = 512                            # points per tile (one PSUM bank)
D = 128
P = 8
K0 = 12
PE_F = 4                              # viewbase_pe
HEADS = 4
DR = 8                                # reduced head dim
DH = 32                               # head dim
DIN_FEAT = 3 + 3 * PE_F * 2 + K0      # 39
EPS = 1e-5

_CACHE = {}


def _np(x):
    return np.asarray(x, dtype=np.float32)


def _prep_block(p, pf, din):
    """Fold one _mlp_block's parameters into kernel matrices.

    Returns dict with A [din,32], bias_l [32], Winc [din,128], Vc [32,128],
    Wf1 [128,256], bf1 [256], Wf2c [256,128], g1s (scalar), Wout_eff
    [128,dout], bout [dout].
    """
    att = p["att"]
    Win = _np(p["Win"]).astype(np.float64)
    bin_ = _np(p["bin"]).astype(np.float64)
    pfp = pf @ _np(p["Winp"]).astype(np.float64) + _np(p["binp"]).astype(np.float64)

    k = (pfp @ _np(att["Wk"]).astype(np.float64) + _np(att["bk"]).astype(np.float64))
    v = (pfp @ _np(att["Wv"]).astype(np.float64) + _np(att["bv"]).astype(np.float64))
    k = k.reshape(P, HEADS, DR)
    v = v.reshape(P, HEADS, DH)
    scale = DH ** -0.5

    Kblk = np.zeros((HEADS * DR, HEADS * P), np.float64)   # [32, 32]
    Vblk = np.zeros((HEADS * P, D), np.float64)            # [32, 128]
    for h in range(HEADS):
        for pp in range(P):
            Kblk[h * DR:(h + 1) * DR, h * P + pp] = k[pp, h] * scale
            Vblk[h * P + pp, h * DH:(h + 1) * DH] = v[pp, h]

    Wq = _np(att["Wq"]).astype(np.float64)
    bq = _np(att["bq"]).astype(np.float64)
    A = Win @ Wq @ Kblk                                    # [din, 32]
    bias_l = (bin_ @ Wq + bq) @ Kblk                       # [32]

    Pc = np.eye(D) - np.full((D, D), 1.0 / D)
    Winc = Win @ Pc                                        # [din, 128]
    bin_c = bin_ @ Pc                                      # [128]
    Vc = Vblk @ Pc + bin_c[None, :] / HEADS                # [32, 128]

    g1 = _np(att["g1"]).astype(np.float64)
    be1 = _np(att["be1"]).astype(np.float64)
    g2 = _np(att["g2"]).astype(np.float64)
    be2 = _np(att["be2"]).astype(np.float64)
    bf2 = _np(att["bf2"]).astype(np.float64)
    assert np.all(be1 == 0), "nonzero be1 needs an extra bias matmul"
    assert np.all(be2 == 0), "nonzero be2 needs an extra bias matmul"
    assert np.all(bf2 == 0), "nonzero bf2 needs an extra bias matmul"
    assert np.ptp(g1) == 0, "non-constant g1 breaks the centered residual fold"
    assert np.all(g2 > 0), "g2 must be positive to commute with relu"
    g1s = float(g1[0])

    Wf1 = g1[:, None] * _np(att["Wf1"]).astype(np.float64)   # [128, 256]
    bf1 = be1 @ _np(att["Wf1"]).astype(np.float64) + _np(att["bf1"]).astype(np.float64)
    Wf2c = _np(att["Wf2"]).astype(np.float64) @ Pc           # [256, 128]
    Wout_eff = g2[:, None] * _np(p["Wout"]).astype(np.float64)
    bout = _np(p["bout"]).astype(np.float64)

    return dict(A=A, bias_l=bias_l, Winc=Winc, Vc=Vc, Wf1=Wf1, bf1=bf1,
                Wf2c=Wf2c, g1s=g1s, Wout_eff=Wout_eff, bout=bout)


def _prep(part, index_mlp, feat_mlp, rgbnets):
    pf = (_np(part["emb"]).astype(np.float64) @ _np(part["W"]).astype(np.float64)
          + _np(part["b"]).astype(np.float64))
    bi = _prep_block(index_mlp, pf, K0)
    bf = _prep_block(feat_mlp, pf, DIN_FEAT)

    W1 = _np(rgbnets["W1"]).astype(np.float64)   # [8,128,128]
    b1 = _np(rgbnets["b1"]).astype(np.float64)
    W2 = _np(rgbnets["W2"]).astype(np.float64)
    b2 = _np(rgbnets["b2"]).astype(np.float64)
    W3 = _np(rgbnets["W3"]).astype(np.float64)   # [8,128,3]
    b3 = _np(rgbnets["b3"]).astype(np.float64)   # [8,3]
    assert np.all(b2 == 0), "nonzero b2 needs ACT relu bias on the W2 stage"

    b1_eff = bf["bout"][None, :] @ W1 + b1[:, None, :]   # [8,1,128] -> squeeze
    b1_eff = b1_eff.reshape(P, D)

    W3Z = np.zeros((D, P * 24), np.float64)
    for pp in range(P):
        W3Z[:, 24 * pp + 3 * pp: 24 * pp + 3 * pp + 3] = W3[pp]
    b3stack = b3.reshape(-1)                     # [24]

    OnesH = np.zeros((32, 4), np.float64)
    SelH = np.zeros((4, 32), np.float64)
    for h in range(HEADS):
        OnesH[h * P:(h + 1) * P, h] = 1.0
        SelH[h, h * P:(h + 1) * P] = 1.0
    R = np.zeros((P, 24), np.float64)
    S = np.zeros((24, 3), np.float64)
    for pp in range(P):
        for c in range(3):
            R[pp, 3 * pp + c] = 1.0
            S[3 * pp + c, c] = 1.0
    Ones83 = np.ones((P, 3), np.float64)

    f32 = lambda x: np.ascontiguousarray(x, dtype=np.float32)

    # packed lhsT weight tensors, grouped by contraction size K
    W12 = f32(np.concatenate([bi["A"], bi["Winc"]], axis=1))            # [12,160]
    W39 = f32(np.concatenate([bf["A"], bf["Winc"]], axis=1))            # [39,160]
    W32 = f32(np.concatenate([OnesH, bi["Vc"], bf["Vc"]], axis=1))      # [32,260]
    W4 = f32(SelH)                                                      # [4,32]
    W8 = f32(np.concatenate([R, Ones83], axis=1))                       # [8,27]
    W24 = f32(S)                                                        # [24,3]

    ones128 = np.full((D, D), 1.0 / D)
    big_cols = [bi["Wf1"][:, :128], bi["Wf1"][:, 128:],
                bi["Wf2c"][:128], bi["Wf2c"][128:],
                bf["Wf1"][:, :128], bf["Wf1"][:, 128:],
                bf["Wf2c"][:128], bf["Wf2c"][128:],
                bi["Wout_eff"],              # [128, 8]
                bf["Wout_eff"],              # [128, 128]
                ones128]
    for pp in range(P):
        big_cols.append(W1[pp])
    for pp in range(P):
        big_cols.append(W2[pp])
    big_cols.append(W3Z)
    WBIG = f32(np.concatenate(big_cols, axis=1))   # [128, 8*128 + 8 + 128 + 128 + 16*128 + 192]

    # column offsets inside WBIG
    off = {}
    c = 0
    for name, w in [("f1a_i", 128), ("f1b_i", 128), ("f2a_i", 128), ("f2b_i", 128),
                    ("f1a_f", 128), ("f1b_f", 128), ("f2a_f", 128), ("f2b_f", 128),
                    ("wout_i", 8), ("wout_f", 128), ("ones", 128)]:
        off[name] = (c, w)
        c += w
    for pp in range(P):
        off[f"w1_{pp}"] = (c, 128)
        c += 128
    for pp in range(P):
        off[f"w2_{pp}"] = (c, 128)
        c += 128
    for pp in range(P):
        off[f"w3_{pp}"] = (c + 24 * pp, 24)
    c += 192
    assert WBIG.shape[1] == c

    # bias pack [128, nb] fp32; each column one bias vector (rows = partitions)
    BIAS = np.zeros((D, 16 + P), np.float32)
    boff = {}
    def put(name, vec):
        j = len(boff)
        BIAS[:len(vec), j] = vec
        boff[name] = j
    put("bias_l_i", f32(bi["bias_l"]))
    put("bias_l_f", f32(bf["bias_l"]))
    put("bf1a_i", f32(bi["bf1"][:128]))
    put("bf1b_i", f32(bi["bf1"][128:]))
    put("bf1a_f", f32(bf["bf1"][:128]))
    put("bf1b_f", f32(bf["bf1"][128:]))
    put("bexp", f32(bi["bout"]))
    put("b3stack", f32(b3stack))
    for pp in range(P):
        put(f"b1_{pp}", f32(b1_eff[pp]))

    return dict(W12=W12, W39=W39, W32=W32, W4=W4, W8=W8, W24=W24, WBIG=WBIG,
                BIAS=BIAS, off=off, boff=boff,
                g1s_i=bi["g1s"], g1s_f=bf["g1s"])


def _build_feat(k0, viewdirs):
    """Host view embedding -> transposed feature matrix [39, N] fp32."""
    n = k0.shape[0]
    freqs = (2.0 ** np.arange(PE_F)).astype(np.float32)
    ve = viewdirs[:, :, None] * freqs                       # [N,3,PE]
    feat = np.empty((DIN_FEAT, n), np.float32)
    feat[0:K0] = k0.T
    feat[K0:K0 + 3] = viewdirs.T
    feat[K0 + 3:K0 + 15] = np.sin(ve).reshape(n, -1).T
    feat[K0 + 15:K0 + 27] = np.cos(ve).reshape(n, -1).T
    return feat


def _build_nc(prep, ntiles):
    import concourse.bass as bass
    import concourse.tile as tile
    from concourse import mybir
    from concourse.alu_op_type import AluOpType

    AF = mybir.ActivationFunctionType
    F32 = mybir.dt.float32
    F32R = mybir.dt.float32r
    off = prep["off"]
    boff = prep["boff"]
    npts = ntiles * TILE

    nc = bass.Bass()
    feat_d = nc.dram_tensor("featT", [DIN_FEAT, npts], F32R, kind="ExternalInput")
    W12_d = nc.dram_tensor("W12", list(prep["W12"].shape), F32R, kind="ExternalInput")
    W39_d = nc.dram_tensor("W39", list(prep["W39"].shape), F32R, kind="ExternalInput")
    W32_d = nc.dram_tensor("W32", list(prep["W32"].shape), F32R, kind="ExternalInput")
    W4_d = nc.dram_tensor("W4", list(prep["W4"].shape), F32R, kind="ExternalInput")
    W8_d = nc.dram_tensor("W8", list(prep["W8"].shape), F32R, kind="ExternalInput")
    W24_d = nc.dram_tensor("W24", list(prep["W24"].shape), F32R, kind="ExternalInput")
    WBIG_d = nc.dram_tensor("WBIG", list(prep["WBIG"].shape), F32R, kind="ExternalInput")
    BIAS_d = nc.dram_tensor("BIAS", list(prep["BIAS"].shape), F32, kind="ExternalInput")
    rgb_d = nc.dram_tensor("rgbT", [3, npts], F32, kind="ExternalOutput")

    with tile.TileContext(nc) as tc:
        with tc.tile_pool(name="const", bufs=1) as cpool, \
             tc.tile_pool(name="work", bufs=2) as work, \
             tc.tile_pool(name="ps", bufs=6, space="PSUM") as pspool:

            def ctile(src, shape, nm, dtype=F32R):
                t = cpool.tile(shape, dtype, name=nm, tag=nm)
                nc.sync.dma_start(t, src)
                return t

            w12 = ctile(W12_d[:, :], list(prep["W12"].shape), "w12")
            w39 = ctile(W39_d[:, :], list(prep["W39"].shape), "w39")
            w32 = ctile(W32_d[:, :], list(prep["W32"].shape), "w32")
            w4 = ctile(W4_d[:, :], list(prep["W4"].shape), "w4")
            w8 = ctile(W8_d[:, :], list(prep["W8"].shape), "w8")
            w24 = ctile(W24_d[:, :], list(prep["W24"].shape), "w24")
            wbig = ctile(WBIG_d[:, :], list(prep["WBIG"].shape), "wbig")
            bias = ctile(BIAS_d[:, :], list(prep["BIAS"].shape), "bias", F32)
            eps_t = cpool.tile([D, 1], F32)
            nc.vector.memset(eps_t, EPS)

            def WB(name):
                c0, w = off[name]
                return wbig[:, c0:c0 + w]

            def BV(name, rows):
                return bias[0:rows, boff[name]:boff[name] + 1]

            ps_cnt = [0]

            def ps(shape, tag="ps", bufs=None):
                ps_cnt[0] += 1
                kw = {"bufs": bufs} if bufs else {}
                return pspool.tile(shape, F32, tag=tag, name=f"ps{ps_cnt[0]}", **kw)

            def block(x0, din, sfx, g1s):
                """One IndexMLP/RelateMLP block on x0 [din, TILE] f32r.
                Returns psO (psum [dout, TILE])."""
                wA = (w12 if din == K0 else w39)
                vc_col = 4 if sfx == "i" else 4 + 128

                psL = ps([32, TILE])
                nc.tensor.matmul(psL, wA[:, 0:32], x0, start=True, stop=True)

                expatt = work.tile([32, TILE], F32R, tag=f"expatt_{sfx}")
                nc.scalar.activation(expatt, psL, AF.Exp,
                                     bias=BV(f"bias_l_{sfx}", 32), scale=1.0)
                psS = ps([4, TILE])
                nc.tensor.matmul(psS, w32[:, 0:4], expatt, start=True, stop=True)
                s4 = work.tile([4, TILE], F32R, tag=f"s4_{sfx}")
                nc.scalar.copy(s4, psS)
                psB = ps([32, TILE])
                nc.tensor.matmul(psB, w4[:, :], s4, start=True, stop=True)
                recB = work.tile([32, TILE], F32, tag=f"recB_{sfx}")
                nc.vector.reciprocal(recB, psB)
                attn = work.tile([32, TILE], F32R, tag=f"attn_{sfx}")
                nc.vector.tensor_tensor(attn, recB, expatt, AluOpType.mult)
                psC = ps([D, TILE], tag="acc", bufs=2)
                nc.tensor.matmul(psC, wA[:, 32:160], x0, start=True, stop=False)
                nc.tensor.matmul(psC, w32[:, vc_col:vc_col + 128], attn,
                                 start=False, stop=True)

                sq = work.tile([D, TILE], F32R, tag=f"sq_{sfx}")
                nc.scalar.square(sq, psC)
                psM = ps([D, TILE])
                nc.tensor.matmul(psM, WB("ones"), sq, start=True, stop=True)
                s = work.tile([D, TILE], F32, tag=f"s_{sfx}")
                nc.scalar.activation(s, psM, AF.Sqrt, bias=eps_t[:, :], scale=1.0)
                rstd = work.tile([D, TILE], F32, tag=f"rstd_{sfx}")
                nc.vector.reciprocal(rstd, s)
                chat = work.tile([D, TILE], F32R, tag=f"chat_{sfx}")
                nc.vector.tensor_tensor(chat, rstd, psC, AluOpType.mult)

                psF1 = ps([D, TILE])
                nc.tensor.matmul(psF1, WB(f"f1a_{sfx}"), chat, start=True, stop=True)
                psF2 = ps([D, TILE])
                nc.tensor.matmul(psF2, WB(f"f1b_{sfx}"), chat, start=True, stop=True)
                r1a = work.tile([D, TILE], F32R, tag=f"r1a_{sfx}")
                nc.scalar.activation(r1a, psF1, AF.Relu,
                                     bias=BV(f"bf1a_{sfx}", D), scale=1.0)
                r1b = work.tile([D, TILE], F32R, tag=f"r1b_{sfx}")
                nc.scalar.activation(r1b, psF2, AF.Relu,
                                     bias=BV(f"bf1b_{sfx}", D), scale=1.0)
                psFF = ps([D, TILE])
                nc.tensor.matmul(psFF, WB(f"f2a_{sfx}"), r1a, start=True, stop=False)
                nc.tensor.matmul(psFF, WB(f"f2b_{sfx}"), r1b, start=False, stop=True)

                c2 = work.tile([D, TILE], F32, tag=f"c2_{sfx}")
                nc.vector.scalar_tensor_tensor(c2, chat, g1s, psFF,
                                               AluOpType.mult, AluOpType.add)
                sq2 = work.tile([D, TILE], F32R, tag=f"sq2_{sfx}")
                nc.scalar.square(sq2, c2)
                psM2 = ps([D, TILE])
                nc.tensor.matmul(psM2, WB("ones"), sq2, start=True, stop=True)
                s2 = work.tile([D, TILE], F32, tag=f"s2_{sfx}")
                nc.scalar.activation(s2, psM2, AF.Sqrt, bias=eps_t[:, :], scale=1.0)
                rstd2 = work.tile([D, TILE], F32, tag=f"rstd2_{sfx}")
                nc.vector.reciprocal(rstd2, s2)
                chat2 = work.tile([D, TILE], F32, tag=f"chat2_{sfx}")
                nc.vector.tensor_tensor(chat2, rstd2, c2, AluOpType.mult)
                r = work.tile([D, TILE], F32R, tag=f"r_{sfx}")
                nc.vector.tensor_relu(r, chat2)

                dout = 8 if sfx == "i" else 128
                psO = ps([dout, TILE])
                nc.tensor.matmul(psO, WB(f"wout_{sfx}"), r, start=True, stop=True)
                return psO

            for t in range(ntiles):
                x0 = work.tile([DIN_FEAT, TILE], F32R, tag="x0")
                nc.sync.dma_start(x0, feat_d[:, t * TILE:(t + 1) * TILE])

                psO_i = block(x0[0:K0, :], K0, "i", prep["g1s_i"])
                expg = work.tile([P, TILE], F32R, tag="expg")
                nc.scalar.activation(expg, psO_i, AF.Exp,
                                     bias=BV("bexp", P), scale=1.0)

                psO_f = block(x0[:, :], DIN_FEAT, "f", prep["g1s_f"])
                h = work.tile([D, TILE], F32R, tag="h")
                nc.scalar.copy(h, psO_f)

                psE = ps([24, TILE], tag="acc", bufs=2)
                for pp in range(P):
                    psE1 = ps([D, TILE])
                    nc.tensor.matmul(psE1, WB(f"w1_{pp}"), h, start=True, stop=True)
                    e1 = work.tile([D, TILE], F32R, tag="e1")
                    nc.scalar.activation(e1, psE1, AF.Relu,
                                         bias=BV(f"b1_{pp}", D), scale=1.0)
                    psE2 = ps([D, TILE])
                    nc.tensor.matmul(psE2, WB(f"w2_{pp}"), e1, start=True, stop=True)
                    e2 = work.tile([D, TILE], F32R, tag="e2")
                    nc.vector.tensor_relu(e2, psE2)
                    nc.tensor.matmul(psE, WB(f"w3_{pp}"), e2,
                                     start=(pp == 0), stop=(pp == P - 1))

                E = work.tile([24, TILE], F32, tag="E")
                nc.scalar.activation(E, psE, AF.Identity,
                                     bias=BV("b3stack", 24), scale=1.0)
                psBC = ps([24, TILE])
                nc.tensor.matmul(psBC, w8[:, 0:24], expg, start=True, stop=True)
                prod = work.tile([24, TILE], F32R, tag="prod")
                nc.vector.tensor_tensor(prod, E, psBC, AluOpType.mult)
                psMix = ps([3, TILE])
                nc.tensor.matmul(psMix, w24[:, :], prod, start=True, stop=True)
                psS3 = ps([3, TILE])
                nc.tensor.matmul(psS3, w8[:, 24:27], expg, start=True, stop=True)
                rc3 = work.tile([3, TILE], F32, tag="rc3")
                nc.vector.reciprocal(rc3, psS3)
                mixn = work.tile([3, TILE], F32, tag="mixn")
                nc.vector.tensor_tensor(mixn, rc3, psMix, AluOpType.mult)
                rgb = work.tile([3, TILE], F32, tag="rgb")
                nc.scalar.activation(rgb, mixn, AF.Sigmoid)
                nc.sync.dma_start(rgb_d[:, t * TILE:(t + 1) * TILE], rgb)

    _install_multiwait_split(nc)
    return nc


def _install_multiwait_split(nc):
    """walrus codegen in this container rejects instructions carrying more
    than one sync wait; split extras onto NoOp carriers at serialization."""
    import json

    orig = nc.to_json_bytes

    def patched():
        d = json.loads(orig())
        for fn in d.get("functions", []):
            for bb in fn.get("blocks", []) or fn.get("basicblocks", []):
                out = []
                for inst in bb.get("instructions", []):
                    si = inst.get("sync_info") or {}
                    waits = si.get("on_wait") or []
                    if len(waits) > 1:
                        for j, w in enumerate(waits[:-1]):
                            out.append({"engine": inst["engine"], "ins": [],
                                        "outs": [],
                                        "name": f"{inst['name']}-wsplit{j}",
                                        "opcode": "NoOp",
                                        "sync_info": {"on_wait": [w],
                                                      "on_update": []}})
                        inst["sync_info"] = {
                            "on_wait": [waits[-1]],
                            "on_update": si.get("on_update") or [],
                        }
                    out.append(inst)
                bb["instructions"] = out
        return json.dumps(d).encode()

    nc.to_json_bytes = patched


def kernel(k0, viewdirs, part, index_mlp, feat_mlp, rgbnets):
    from concourse.bass_utils import run_bass_kernel_spmd

    k0 = _np(k0)
    viewdirs = _np(viewdirs)

    ntiles = int(os.environ.get("KERNEL_TILES", NP_CORE // TILE))
    npts = ntiles * TILE

    key = ("nc", ntiles)
    if key not in _CACHE:
        prep = _prep(part, index_mlp, feat_mlp, rgbnets)
        nc = _build_nc(prep, ntiles)
        _CACHE[key] = (nc, prep)
    nc, prep = _CACHE[key]

    featT = _build_feat(k0, viewdirs)   # [39, N]

    weight_map = {k: prep[k] for k in
                  ["W12", "W39", "W32", "W4", "W8", "W24", "WBIG", "BIAS"]}
    in_maps = []
    for c in range(NCORES):
        m = dict(weight_map)
        m["featT"] = np.ascontiguousarray(
            featT[:, c * NP_CORE:c * NP_CORE + npts])
        in_maps.append(m)

    res = run_bass_kernel_spmd(nc, in_maps, core_ids=list(range(NCORES)))
    outs = [r["rgbT"] for r in res.results]      # each [3, npts]

    rgb = np.empty((N_TOTAL, 3), np.float32)
    for c in range(NCORES):
        rgb[c * NP_CORE:c * NP_CORE + npts] = outs[c].T
        if npts < NP_CORE:
            rgb[c * NP_CORE + npts:(c + 1) * NP_CORE] = 0
    return rgb


# revision 11
# speedup vs baseline: 339.7668x; 339.7668x over previous
"""Trainium2 Bass kernel for the NeRF MoE-routing module.

Strategy
--------
Data-parallel over points N=131072 across 8 NeuronCores (16384 points/core).
On-chip layout is transposed: activations live as [features, points] with
features on SBUF partitions, so every linear layer is one PE matmul
(out = lhsT.T @ rhs) and layers chain without transposes.

Host-side folding (all free — done once in numpy):
  * part features pf, their attention projections k/v, and the per-head
    block-diagonal attention matrices (Kblk/Vblk) are precomputed.
  * the q-projection is folded through Win into a single [din,32] matrix.
  * LayerNorm mean subtraction is folded into centered weights (P = I-11^T/128)
    so the kernel only computes the rstd scaling.
  * the view embedding (sin/cos) is precomputed into a [39, N] feature matrix.
Matmuls run in float32r (full-speed fp32-storage mode, ~1e-4 relative error).
"""
import os
import sys

sys.path.insert(0, "/opt/trn_rl_repo")

import numpy as np

N_TOTAL = 131072
NCORES = 8
NP_CORE = N_TOTAL // NCORES          # 16384 points per core
TILE = 512                            # points per tile (one PSUM bank)
D = 128
P = 8
K0 = 12
PE_F = 4                              # viewbase_pe
HEADS = 4
DR = 8                                # reduced head dim
DH = 32                               # head dim
DIN_FEAT = 3 + 3 * PE_F * 2 + K0      # 39
EPS = 1e-5

_CACHE = {}


def _np(x):
    return np.asarray(x, dtype=np.float32)


def _prep_block(p, pf, din):
    """Fold one _mlp_block's parameters into kernel matrices.

    Returns dict with A [din,32], bias_l [32], Winc [din,128], Vc [32,128],
    Wf1 [128,256], bf1 [256], Wf2c [256,128], g1s (scalar), Wout_eff
    [128,dout], bout [dout].
    """
    att = p["att"]
    Win = _np(p["Win"]).astype(np.float64)
    bin_ = _np(p["bin"]).astype(np.float64)
    pfp = pf @ _np(p["Winp"]).astype(np.float64) + _np(p["binp"]).astype(np.float64)

    k = (pfp @ _np(att["Wk"]).astype(np.float64) + _np(att["bk"]).astype(np.float64))
    v = (pfp @ _np(att["Wv"]).astype(np.float64) + _np(att["bv"]).astype(np.float64))
    k = k.reshape(P, HEADS, DR)
    v = v.reshape(P, HEADS, DH)
    scale = DH ** -0.5

    Kblk = np.zeros((HEADS * DR, HEADS * P), np.float64)   # [32, 32]
    Vblk = np.zeros((HEADS * P, D), np.float64)            # [32, 128]
    for h in range(HEADS):
        for pp in range(P):
            Kblk[h * DR:(h + 1) * DR, h * P + pp] = k[pp, h] * scale
            Vblk[h * P + pp, h * DH:(h + 1) * DH] = v[pp, h]

    Wq = _np(att["Wq"]).astype(np.float64)
    bq = _np(att["bq"]).astype(np.float64)
    A = Win @ Wq @ Kblk                                    # [din, 32]
    bias_l = (bin_ @ Wq + bq) @ Kblk                       # [32]

    Pc = np.eye(D) - np.full((D, D), 1.0 / D)
    Winc = Win @ Pc                                        # [din, 128]
    bin_c = bin_ @ Pc                                      # [128]
    Vc = Vblk @ Pc + bin_c[None, :] / HEADS                # [32, 128]

    g1 = _np(att["g1"]).astype(np.float64)
    be1 = _np(att["be1"]).astype(np.float64)
    g2 = _np(att["g2"]).astype(np.float64)
    be2 = _np(att["be2"]).astype(np.float64)
    bf2 = _np(att["bf2"]).astype(np.float64)
    assert np.all(be1 == 0), "nonzero be1 needs an extra bias matmul"
    assert np.all(be2 == 0), "nonzero be2 needs an extra bias matmul"
    assert np.all(bf2 == 0), "nonzero bf2 needs an extra bias matmul"
    assert np.ptp(g1) == 0, "non-constant g1 breaks the centered residual fold"
    assert np.all(g2 > 0), "g2 must be positive to commute with relu"
    g1s = float(g1[0])

    Wf1 = g1[:, None] * _np(att["Wf1"]).astype(np.float64)   # [128, 256]
    bf1 = be1 @ _np(att["Wf1"]).astype(np.float64) + _np(att["bf1"]).astype(np.float64)
    Wf2c = _np(att["Wf2"]).astype(np.float64) @ Pc           # [256, 128]
    Wout_eff = g2[:, None] * _np(p["Wout"]).astype(np.float64)
    bout = _np(p["bout"]).astype(np.float64)

    return dict(A=A, bias_l=bias_l, Winc=Winc, Vc=Vc, Wf1=Wf1, bf1=bf1,
                Wf2c=Wf2c, g1s=g1s, Wout_eff=Wout_eff, bout=bout)


def _prep(part, index_mlp, feat_mlp, rgbnets):
    pf = (_np(part["emb"]).astype(np.float64) @ _np(part["W"]).astype(np.float64)
          + _np(part["b"]).astype(np.float64))
    bi = _prep_block(index_mlp, pf, K0)
    bf = _prep_block(feat_mlp, pf, DIN_FEAT)

    W1 = _np(rgbnets["W1"]).astype(np.float64)   # [8,128,128]
    b1 = _np(rgbnets["b1"]).astype(np.float64)
    W2 = _np(rgbnets["W2"]).astype(np.float64)
    b2 = _np(rgbnets["b2"]).astype(np.float64)
    W3 = _np(rgbnets["W3"]).astype(np.float64)   # [8,128,3]
    b3 = _np(rgbnets["b3"]).astype(np.float64)   # [8,3]
    assert np.all(b2 == 0), "nonzero b2 needs ACT relu bias on the W2 stage"

    b1_eff = bf["bout"][None, :] @ W1 + b1[:, None, :]   # [8,1,128] -> squeeze
    b1_eff = b1_eff.reshape(P, D)

    W3Z = np.zeros((D, P * 24), np.float64)
    for pp in range(P):
        W3Z[:, 24 * pp + 3 * pp: 24 * pp + 3 * pp + 3] = W3[pp]
    b3stack = b3.reshape(-1)                     # [24]

    OnesH = np.zeros((32, 4), np.float64)
    SelH = np.zeros((4, 32), np.float64)
    for h in range(HEADS):
        OnesH[h * P:(h + 1) * P, h] = 1.0
        SelH[h, h * P:(h + 1) * P] = 1.0
    R = np.zeros((P, 24), np.float64)
    S = np.zeros((24, 3), np.float64)
    for pp in range(P):
        for c in range(3):
            R[pp, 3 * pp + c] = 1.0
            S[3 * pp + c, c] = 1.0
    Ones83 = np.ones((P, 3), np.float64)

    f32 = lambda x: np.ascontiguousarray(x, dtype=np.float32)

    # packed lhsT weight tensors, grouped by contraction size K
    W12 = f32(np.concatenate([bi["A"], bi["Winc"]], axis=1))            # [12,160]
    W39 = f32(np.concatenate([bf["A"], bf["Winc"]], axis=1))            # [39,160]
    W32 = f32(np.concatenate([OnesH, bi["Vc"], bf["Vc"]], axis=1))      # [32,260]
    W4 = f32(SelH)                                                      # [4,32]
    W8 = f32(np.concatenate([R, Ones83], axis=1))                       # [8,27]
    W24 = f32(S)                                                        # [24,3]

    ones128 = np.full((D, D), 1.0 / D)
    big_cols = [bi["Wf1"][:, :128], bi["Wf1"][:, 128:],
                bi["Wf2c"][:128], bi["Wf2c"][128:],
                bf["Wf1"][:, :128], bf["Wf1"][:, 128:],
                bf["Wf2c"][:128], bf["Wf2c"][128:],
                bi["Wout_eff"],              # [128, 8]
                bf["Wout_eff"],              # [128, 128]
                ones128]
    for pp in range(P):
        big_cols.append(W1[pp])
    for pp in range(P):
        big_cols.append(W2[pp])
    big_cols.append(W3Z)
    WBIG = f32(np.concatenate(big_cols, axis=1))   # [128, 8*128 + 8 + 128 + 128 + 16*128 + 192]

    # column offsets inside WBIG
    off = {}
    c = 0
    for name, w in [("f1a_i", 128), ("f1b_i", 128), ("f2a_i", 128), ("f2b_i", 128),
                    ("f1a_f", 128), ("f1b_f", 128), ("f2a_f", 128), ("f2b_f", 128),
                    ("wout_i", 8), ("wout_f", 128), ("ones", 128)]:
        off[name] = (c, w)
        c += w
    for pp in range(P):
        off[f"w1_{pp}"] = (c, 128)
        c += 128
    for pp in range(P):
        off[f"w2_{pp}"] = (c, 128)
        c += 128
    for pp in range(P):
        off[f"w3_{pp}"] = (c + 24 * pp, 24)
    c += 192
    assert WBIG.shape[1] == c

    # bias pack [128, nb] fp32; each column one bias vector (rows = partitions)
    BIAS = np.zeros((D, 16 + P), np.float32)
    boff = {}
    def put(name, vec):
        j = len(boff)
        BIAS[:len(vec), j] = vec
        boff[name] = j
    put("bias_l_i", f32(bi["bias_l"]))
    put("bias_l_f", f32(bf["bias_l"]))
    put("bf1a_i", f32(bi["bf1"][:128]))
    put("bf1b_i", f32(bi["bf1"][128:]))
    put("bf1a_f", f32(bf["bf1"][:128]))
    put("bf1b_f", f32(bf["bf1"][128:]))
    put("bexp", f32(bi["bout"]))
    put("b3stack", f32(b3stack))
    for pp in range(P):
        put(f"b1_{pp}", f32(b1_eff[pp]))

    return dict(W12=W12, W39=W39, W32=W32, W4=W4, W8=W8, W24=W24, WBIG=WBIG,
                BIAS=BIAS, off=off, boff=boff,
                g1s_i=bi["g1s"], g1s_f=bf["g1s"],
                b1_zero=bool(np.all(b1_eff == 0)))


def _build_feat(k0, viewdirs):
    """Host view embedding -> transposed feature matrix [39, N] fp32."""
    n = k0.shape[0]
    freqs = (2.0 ** np.arange(PE_F)).astype(np.float32)
    ve = viewdirs[:, :, None] * freqs                       # [N,3,PE]
    feat = np.empty((DIN_FEAT, n), np.float32)
    feat[0:K0] = k0.T
    feat[K0:K0 + 3] = viewdirs.T
    feat[K0 + 3:K0 + 15] = np.sin(ve).reshape(n, -1).T
    feat[K0 + 15:K0 + 27] = np.cos(ve).reshape(n, -1).T
    return feat


def _build_nc(prep, ntiles):
    abl = set(os.environ.get("KERNEL_ABL", "").split(","))
    import concourse.bass as bass
    import concourse.tile as tile
    from concourse import mybir
    from concourse.alu_op_type import AluOpType

    AF = mybir.ActivationFunctionType
    F32 = mybir.dt.float32
    F32R = mybir.dt.float32r
    off = prep["off"]
    boff = prep["boff"]
    npts = ntiles * TILE

    nc = bass.Bass()
    feat_d = nc.dram_tensor("featT", [DIN_FEAT, npts], F32R, kind="ExternalInput")
    W12_d = nc.dram_tensor("W12", list(prep["W12"].shape), F32R, kind="ExternalInput")
    W39_d = nc.dram_tensor("W39", list(prep["W39"].shape), F32R, kind="ExternalInput")
    W32_d = nc.dram_tensor("W32", list(prep["W32"].shape), F32R, kind="ExternalInput")
    W4_d = nc.dram_tensor("W4", list(prep["W4"].shape), F32R, kind="ExternalInput")
    W8_d = nc.dram_tensor("W8", list(prep["W8"].shape), F32R, kind="ExternalInput")
    W24_d = nc.dram_tensor("W24", list(prep["W24"].shape), F32R, kind="ExternalInput")
    WBIG_d = nc.dram_tensor("WBIG", list(prep["WBIG"].shape), F32R, kind="ExternalInput")
    BIAS_d = nc.dram_tensor("BIAS", list(prep["BIAS"].shape), F32, kind="ExternalInput")
    mix_d = nc.dram_tensor("mixT", [3, npts], F32, kind="ExternalOutput")
    sum_d = nc.dram_tensor("sumT", [1, npts], F32, kind="ExternalOutput")

    with tile.TileContext(nc) as tc:
        with tc.tile_pool(name="const", bufs=1) as cpool, \
             tc.tile_pool(name="work", bufs=2) as work, \
             tc.tile_pool(name="ps", bufs=6, space="PSUM") as pspool:

            def ctile(src, shape, nm, dtype=F32R):
                t = cpool.tile(shape, dtype, name=nm, tag=nm)
                nc.sync.dma_start(t, src)
                return t

            w12 = ctile(W12_d[:, :], list(prep["W12"].shape), "w12")
            w39 = ctile(W39_d[:, :], list(prep["W39"].shape), "w39")
            w32 = ctile(W32_d[:, :], list(prep["W32"].shape), "w32")
            w4 = ctile(W4_d[:, :], list(prep["W4"].shape), "w4")
            w8 = ctile(W8_d[:, :], list(prep["W8"].shape), "w8")
            w24 = ctile(W24_d[:, :], list(prep["W24"].shape), "w24")
            wbig = ctile(WBIG_d[:, :], list(prep["WBIG"].shape), "wbig")
            bias = ctile(BIAS_d[:, :], list(prep["BIAS"].shape), "bias", F32)
            eps_t = cpool.tile([D, 1], F32)
            nc.vector.memset(eps_t, EPS)

            def WB(name):
                c0, w = off[name]
                return wbig[:, c0:c0 + w]

            def BV(name, rows):
                return bias[0:rows, boff[name]:boff[name] + 1]

            ps_cnt = [0]

            def ps(shape, tag="ps", bufs=None):
                ps_cnt[0] += 1
                kw = {"bufs": bufs} if bufs else {}
                return pspool.tile(shape, F32, tag=tag, name=f"ps{ps_cnt[0]}", **kw)

            def block(x0, din, sfx, g1s):
                """One IndexMLP/RelateMLP block on x0 [din, TILE] f32r.
                Returns psO (psum [dout, TILE])."""
                wA = (w12 if din == K0 else w39)
                vc_col = 4 if sfx == "i" else 4 + 128

                psL = ps([32, TILE])
                nc.tensor.matmul(psL, wA[:, 0:32], x0, start=True, stop=True)

                expatt = work.tile([32, TILE], F32R, tag=f"expatt_{sfx}")
                nc.scalar.activation(expatt, psL, AF.Exp,
                                     bias=BV(f"bias_l_{sfx}", 32), scale=1.0)
                if "noattn" in abl:
                    attn = expatt
                else:
                    psS = ps([4, TILE])
                    nc.tensor.matmul(psS, w32[:, 0:4], expatt, start=True, stop=True)
                    lse4 = work.tile([4, TILE], F32, tag=f"lse4_{sfx}")
                    nc.scalar.activation(lse4, psS, AF.Ln)
                    rec4 = work.tile([4, TILE], F32R, tag=f"rec4_{sfx}")
                    nc.scalar.activation(rec4, lse4, AF.Exp, scale=-1.0)
                    psB = ps([32, TILE])
                    nc.tensor.matmul(psB, w4[:, :], rec4, start=True, stop=True)
                    attn = work.tile([32, TILE], F32R, tag=f"attn_{sfx}")
                    nc.vector.tensor_tensor(attn, psB, expatt, AluOpType.mult)
                psC = ps([D, TILE], tag="acc", bufs=2)
                nc.tensor.matmul(psC, wA[:, 32:160], x0, start=True, stop=False)
                nc.tensor.matmul(psC, w32[:, vc_col:vc_col + 128], attn,
                                 start=False, stop=True)

                chat = work.tile([D, TILE], F32R, tag=f"chat_{sfx}")
                if "noln" in abl:
                    nc.vector.tensor_copy(chat, psC)
                else:
                    sq = work.tile([D, TILE], F32R, tag=f"sq_{sfx}")
                    nc.scalar.square(sq, psC)
                    psM = ps([D, TILE])
                    nc.tensor.matmul(psM, WB("ones"), sq, start=True, stop=True)
                    lnv = work.tile([D, TILE], F32, tag=f"lnv_{sfx}")
                    nc.scalar.activation(lnv, psM, AF.Ln, bias=eps_t[:, :], scale=1.0)
                    rstd = work.tile([D, TILE], F32, tag=f"rstd_{sfx}")
                    nc.scalar.activation(rstd, lnv, AF.Exp, scale=-0.5)
                    nc.vector.tensor_tensor(chat, rstd, psC, AluOpType.mult)

                if "noff" in abl:
                    r = work.tile([D, TILE], F32R, tag=f"r_{sfx}")
                    nc.vector.tensor_relu(r, chat)
                    dout = 8 if sfx == "i" else 128
                    psO = ps([dout, TILE])
                    nc.tensor.matmul(psO, WB(f"wout_{sfx}"), r, start=True, stop=True)
                    return psO
                psF1 = ps([D, TILE])
                nc.tensor.matmul(psF1, WB(f"f1a_{sfx}"), chat, start=True, stop=True)
                psF2 = ps([D, TILE])
                nc.tensor.matmul(psF2, WB(f"f1b_{sfx}"), chat, start=True, stop=True)
                r1a = work.tile([D, TILE], F32R, tag=f"r1a_{sfx}")
                nc.scalar.activation(r1a, psF1, AF.Relu,
                                     bias=BV(f"bf1a_{sfx}", D), scale=1.0)
                r1b = work.tile([D, TILE], F32R, tag=f"r1b_{sfx}")
                nc.scalar.activation(r1b, psF2, AF.Relu,
                                     bias=BV(f"bf1b_{sfx}", D), scale=1.0)
                psFF = ps([D, TILE])
                nc.tensor.matmul(psFF, WB(f"f2a_{sfx}"), r1a, start=True, stop=False)
                nc.tensor.matmul(psFF, WB(f"f2b_{sfx}"), r1b, start=False, stop=True)

                c2 = work.tile([D, TILE], F32, tag=f"c2_{sfx}")
                nc.vector.scalar_tensor_tensor(c2, chat, g1s, psFF,
                                               AluOpType.mult, AluOpType.add)
                sq2 = work.tile([D, TILE], F32R, tag=f"sq2_{sfx}")
                nc.vector.tensor_tensor(sq2, c2, c2, AluOpType.mult)
                psM2 = ps([D, TILE])
                nc.tensor.matmul(psM2, WB("ones"), sq2, start=True, stop=True)
                lnv2 = work.tile([D, TILE], F32, tag=f"lnv2_{sfx}")
                nc.scalar.activation(lnv2, psM2, AF.Ln, bias=eps_t[:, :], scale=1.0)
                rstd2 = work.tile([D, TILE], F32, tag=f"rstd2_{sfx}")
                nc.scalar.activation(rstd2, lnv2, AF.Exp, scale=-0.5)
                chat2 = work.tile([D, TILE], F32, tag=f"chat2_{sfx}")
                nc.vector.tensor_tensor(chat2, rstd2, c2, AluOpType.mult)
                r = work.tile([D, TILE], F32R, tag=f"r_{sfx}")
                nc.vector.tensor_relu(r, chat2)

                dout = 8 if sfx == "i" else 128
                psO = ps([dout, TILE])
                nc.tensor.matmul(psO, WB(f"wout_{sfx}"), r, start=True, stop=True)
                return psO

            reps = int(os.environ.get("KERNEL_REPS", "1"))
            for t in [tt for _ in range(reps) for tt in range(ntiles)]:
                x0 = work.tile([DIN_FEAT, TILE], F32R, tag="x0")
                nc.sync.dma_start(x0, feat_d[:, t * TILE:(t + 1) * TILE])

                psO_i = block(x0[0:K0, :], K0, "i", prep["g1s_i"])
                expg = work.tile([P, TILE], F32R, tag="expg")
                nc.scalar.activation(expg, psO_i, AF.Exp,
                                     bias=BV("bexp", P), scale=1.0)

                psO_f = block(x0[:, :], DIN_FEAT, "f", prep["g1s_f"])
                h = work.tile([D, TILE], F32R, tag="h")
                nc.vector.tensor_copy(h, psO_f)
                if "noexp" in abl:
                    mixo = work.tile([3, TILE], F32, tag="mixo")
                    nc.scalar.copy(mixo, psO_f[0:3, :])
                    nc.sync.dma_start(mix_d[:, t * TILE:(t + 1) * TILE], mixo)
                    continue

                psE = ps([24, TILE], tag="acc", bufs=2)
                for pp in range(P):
                    psE1 = ps([D, TILE])
                    nc.tensor.matmul(psE1, WB(f"w1_{pp}"), h, start=True, stop=True)
                    e1 = work.tile([D, TILE], F32R, tag="e1")
                    if prep["b1_zero"] and pp % 2 == 1:
                        nc.vector.tensor_relu(e1, psE1)
                    else:
                        nc.scalar.activation(e1, psE1, AF.Relu,
                                             bias=BV(f"b1_{pp}", D), scale=1.0)
                    psE2 = ps([D, TILE])
                    nc.tensor.matmul(psE2, WB(f"w2_{pp}"), e1, start=True, stop=True)
                    e2 = work.tile([D, TILE], F32R, tag="e2")
                    nc.vector.tensor_relu(e2, psE2)
                    nc.tensor.matmul(psE, WB(f"w3_{pp}"), e2,
                                     start=(pp == 0), stop=(pp == P - 1))

                E = work.tile([24, TILE], F32, tag="E")
                nc.scalar.activation(E, psE, AF.Identity,
                                     bias=BV("b3stack", 24), scale=1.0)
                psBC = ps([24, TILE])
                nc.tensor.matmul(psBC, w8[:, 0:24], expg, start=True, stop=True)
                prod = work.tile([24, TILE], F32R, tag="prod")
                nc.vector.tensor_tensor(prod, E, psBC, AluOpType.mult)
                psMix = ps([3, TILE])
                nc.tensor.matmul(psMix, w24[:, :], prod, start=True, stop=True)
                psS3 = ps([3, TILE])
                nc.tensor.matmul(psS3, w8[:, 24:27], expg, start=True, stop=True)
                mixo = work.tile([3, TILE], F32, tag="mixo")
                nc.scalar.copy(mixo, psMix)
                sumo = work.tile([1, TILE], F32, tag="sumo")
                nc.scalar.copy(sumo, psS3[0:1, :])
                nc.sync.dma_start(mix_d[:, t * TILE:(t + 1) * TILE], mixo)
                nc.sync.dma_start(sum_d[:, t * TILE:(t + 1) * TILE], sumo)

    _install_multiwait_split(nc)
    return nc


def _install_multiwait_split(nc):
    """walrus codegen in this container rejects instructions carrying more
    than one sync wait; split extras onto NoOp carriers at serialization."""
    import json

    orig = nc.to_json_bytes

    def patched():
        d = json.loads(orig())
        for fn in d.get("functions", []):
            for bb in fn.get("blocks", []) or fn.get("basicblocks", []):
                out = []
                for inst in bb.get("instructions", []):
                    si = inst.get("sync_info") or {}
                    waits = si.get("on_wait") or []
                    if len(waits) > 1:
                        for j, w in enumerate(waits[:-1]):
                            out.append({"engine": inst["engine"], "ins": [],
                                        "outs": [],
                                        "name": f"{inst['name']}-wsplit{j}",
                                        "opcode": "NoOp",
                                        "sync_info": {"on_wait": [w],
                                                      "on_update": []}})
                        inst["sync_info"] = {
                            "on_wait": [waits[-1]],
                            "on_update": si.get("on_update") or [],
                        }
                    out.append(inst)
                bb["instructions"] = out
        return json.dumps(d).encode()

    nc.to_json_bytes = patched


def kernel(k0, viewdirs, part, index_mlp, feat_mlp, rgbnets):
    from concourse.bass_utils import run_bass_kernel_spmd

    k0 = _np(k0)
    viewdirs = _np(viewdirs)

    ntiles = int(os.environ.get("KERNEL_TILES", NP_CORE // TILE))
    npts = ntiles * TILE

    key = ("nc", ntiles, os.environ.get("KERNEL_REPS", "1"))
    if key not in _CACHE:
        prep = _prep(part, index_mlp, feat_mlp, rgbnets)
        nc = _build_nc(prep, ntiles)
        _CACHE[key] = (nc, prep)
    nc, prep = _CACHE[key]

    featT = _build_feat(k0, viewdirs)   # [39, N]

    weight_map = {k: prep[k] for k in
                  ["W12", "W39", "W32", "W4", "W8", "W24", "WBIG", "BIAS"]}
    in_maps = []
    for c in range(NCORES):
        m = dict(weight_map)
        m["featT"] = np.ascontiguousarray(
            featT[:, c * NP_CORE:c * NP_CORE + npts])
        in_maps.append(m)

    res = run_bass_kernel_spmd(nc, in_maps, core_ids=list(range(NCORES)))

    rgb = np.empty((N_TOTAL, 3), np.float32)
    for c in range(NCORES):
        mix = res.results[c]["mixT"]             # [3, npts]
        sm = res.results[c]["sumT"]              # [1, npts]
        z = (mix / sm).T.astype(np.float32)
        rgb[c * NP_CORE:c * NP_CORE + npts] = 1.0 / (1.0 + np.exp(-z))
        if npts < NP_CORE:
            rgb[c * NP_CORE + npts:(c + 1) * NP_CORE] = 0
    return rgb
